# revision 11
# baseline (speedup 1.0000x reference)
"""Trainium2 Bass kernel for a 2-layer bidirectional LSTM encoder.

Model (matches the reference):
  x = emb[idc]                      # [B=256, T=128, E=256]
  y0 = biLSTM_0(x)                  # H=256 per direction
  y1 = biLSTM_1(y0)
  out = y1[last timestep]           # [256, 512]

Sharding: data-parallel over the 256 utterances, 32 per NeuronCore, no
collectives.  Weights/embedding are replicated.  Structural shortcuts:
  - layer-1 backward only needs ONE step (output keeps position T-1, which is
    the first step of the reversed scan, from zero state).
  - layer-1 forward needs the full chain.

Per-core device program (gate-major layout: gate/hidden dims on partitions,
batch on the free axis, so no per-step transpose is needed):
  A) embedding gather (indirect DMA) -> cast bf16 -> PE transpose -> X.T;
     batched input projections for layer-0 f/b (PSUM -> +bias -> SBUF spans);
     128 interleaved steps of the l0f and l0b recurrences; h-seqs kept in
     SBUF (Y0f / Y0b); layer-1 fwd input projection (h0f part) batched into
     DRAM as it becomes available.
  B) layer-1 fwd chain: recurrent matmuls + (fpart-from-DRAM + bpart-batched)
     projections merged; then the single layer-1 bwd step; output staging.

Matmuls/weights/hidden in bf16 (fp32 PSUM accumulate); cell state c in fp32.
"""

import os
import sys

import numpy as np

for _p in ("/opt/trn_rl_repo",):
    if _p not in sys.path and os.path.isdir(_p):
        sys.path.insert(0, _p)

import ml_dtypes
from contextlib import ExitStack

import concourse.bacc as bacc
import concourse.bass as bass
import concourse.mybir as mybir
import concourse.tile as tile
from concourse.bass import IndirectOffsetOnAxis
from concourse.bass_utils import run_bass_kernel_spmd
from concourse.masks import make_identity

F32 = mybir.dt.float32
BF16 = mybir.dt.bfloat16
I32 = mybir.dt.int32
AF = mybir.ActivationFunctionType

V, E, H = 50257, 256, 256
NUM_UTT = 256
N_CORES = 8
BC = NUM_UTT // N_CORES  # 32 utterances per core
T_FULL = 128

bf16 = ml_dtypes.bfloat16


# ---------------------------------------------------------------- host prep

def _perm_rows(w):
    # PyTorch gate order i,f,g,o (blocks of H rows) -> i,f,o,g
    i, f, g, o = (w[k * H:(k + 1) * H] for k in range(4))
    return np.concatenate([i, f, o, g], 0)


def _prep_weights(inputs):
    """Transpose/permute weights on the host (layout only, no math)."""
    out = {}
    for key in ("w_ih0f", "w_hh0f", "w_ih0b", "w_hh0b",
                "w_ih1f", "w_hh1f", "w_ih1b"):
        w = np.asarray(inputs[key], np.float32)
        out[key] = np.ascontiguousarray(_perm_rows(w).T).astype(bf16)
    for key in ("b0f", "b0b", "b1f", "b1b"):
        b = np.asarray(inputs[key], np.float32)
        bp = _perm_rows(b.reshape(4 * H, 1)).reshape(4 * H)
        out[key] = np.ascontiguousarray(bp.reshape(8, 128).T).astype(np.float32)
    return out


def _reorder_idx(idc_c, T):
    # [BC, T] -> [128, CH]; gather chunk g covers timesteps [TPC*g, TPC*(g+1))
    # for all BC utterances, position p = (t - TPC*g)*BC + (u)
    CH = T * BC // 128
    a = np.ascontiguousarray(idc_c.T).reshape(CH, 128).T  # [128, CH]
    return np.ascontiguousarray(a).astype(np.int32)


# ---------------------------------------------------------------- device IR

def build_program(T):
    assert T % 16 == 0 and 128 % BC == 0
    CH = T * BC // 128       # gather chunks (4 timesteps each)
    SP = T // 16             # 16-step spans
    SPC = 16 * 8 * BC        # columns per span: (t, m, u) = 4096
    GB = 8 * BC              # gate columns per step = 256

    nc = bacc.Bacc("TRN2", target_bir_lowering=False, debug=False)

    idxr_d = nc.declare_dram_parameter("idxr", [128, CH], I32, isOutput=False)
    emb_d = nc.declare_dram_parameter("emb", [V, E], F32, isOutput=False)
    wd = {}
    for key, shape in (
        ("w_ih0f", [E, 1024]), ("w_hh0f", [H, 1024]),
        ("w_ih0b", [E, 1024]), ("w_hh0b", [H, 1024]),
        ("w_ih1f", [2 * H, 1024]), ("w_hh1f", [H, 1024]),
        ("w_ih1b", [2 * H, 1024]),
    ):
        wd[key] = nc.declare_dram_parameter(key, shape, BF16, isOutput=False)
    for key in ("b0f", "b0b", "b1f", "b1b"):
        wd[key] = nc.declare_dram_parameter(key, [128, 8], F32, isOutput=False)
    out_d = nc.declare_dram_parameter("out", [128, 4 * BC], F32, isOutput=True)

    p1f_dram = nc.dram_tensor("p1f_dram", [128, T * GB], BF16)

    with tile.TileContext(nc) as tc, ExitStack() as octx:
        const = octx.enter_context(tc.tile_pool(name="const", bufs=1))

        ident = const.tile([128, 128], BF16)
        make_identity(nc, ident[:])

        idx_sb = const.tile([128, CH], I32)
        nc.sync.dma_start(out=idx_sb[:], in_=idxr_d.ap())

        wsb = {}
        for key, kt in (("w_ih0f", 2), ("w_hh0f", 2), ("w_ih0b", 2),
                        ("w_hh0b", 2), ("w_ih1f", 4), ("w_hh1f", 2),
                        ("w_ih1b", 4)):
            wsb[key] = const.tile([128, kt, 1024], BF16, name=key, tag=key)
            nc.sync.dma_start(
                out=wsb[key][:],
                in_=wd[key].ap().rearrange("(k p) n -> p k n", p=128))
        for key in ("b0f", "b0b", "b1f", "b1b"):
            wsb[key] = const.tile([128, 8], F32, name=key, tag=key)
            nc.sync.dma_start(out=wsb[key][:], in_=wd[key].ap())

        # h sequences of layer 0, both dirs; cols = t*64 + k*32 + u
        y0f = const.tile([128, T * 64], BF16)
        y0b = const.tile([128, T * 64], BF16)
        out_sb = const.tile([128, 4 * BC], F32)

        # -------------------------------------------------- helpers
        def emit_recur(rpool, whh, h_prev, xsl):
            """gates PSUM = xsl (via identity matmul) + W_hh @ h_prev.

            The identity matmul has no dependency on h_prev, so it can be
            scheduled during the previous step's activation/cell tail.
            """
            ps = rpool.tile([128, GB], F32)
            nc.tensor.matmul(ps[:], lhsT=ident[:], rhs=xsl,
                             start=True, stop=False)
            for m in range(8):
                for k in range(2):
                    nc.tensor.matmul(
                        ps[:, BC * m:BC * (m + 1)],
                        lhsT=whh[:, k, 128 * m:128 * (m + 1)],
                        rhs=h_prev[:, 32 * k:32 * (k + 1)],
                        start=False, stop=(m == 7 and k == 1))
            return ps

        def emit_cell(pools, ps, xsl, c_prev, h_out):
            """One LSTM cell update in gate-major layout.

            ps: [128, GB] f32 PSUM gates (x-proj + recurrent), or None at
                step 0 (then xsl already is the full pre-activation)
            h_out: [128, 64] destination AP for the new hidden state
            returns the new cell state tile [128, 64] f32
            """
            gpool, spool, dpool, cpool = pools
            g_ap = xsl if ps is None else ps
            sg = spool.tile([128, 6 * BC], BF16, tag="sg")
            nc.scalar.activation(sg[:], g_ap[:, :6 * BC], AF.Sigmoid)
            tg = spool.tile([128, 2 * BC], BF16, tag="tg")
            nc.scalar.activation(tg[:], g_ap[:, 6 * BC:8 * BC], AF.Tanh)
            c_new = cpool.tile([128, 2 * BC], F32)
            if c_prev is None:
                t1 = dpool.tile([128, 2 * BC], BF16, tag="t1")
                nc.vector.tensor_mul(t1[:], sg[:, :2 * BC], tg[:])
                nc.vector.tensor_copy(c_new[:], t1[:])
            else:
                u = dpool.tile([128, 2 * BC], F32, tag="u")
                nc.vector.tensor_mul(u[:], sg[:, 2 * BC:4 * BC], c_prev[:])
                t1 = dpool.tile([128, 2 * BC], BF16, tag="t1")
                nc.vector.tensor_mul(t1[:], sg[:, :2 * BC], tg[:])
                nc.vector.tensor_add(c_new[:], u[:], t1[:])
            tc_ = dpool.tile([128, 2 * BC], BF16, tag="tc")
            nc.scalar.activation(tc_[:], c_new[:], AF.Tanh)
            nc.vector.tensor_mul(h_out, sg[:, 4 * BC:6 * BC], tc_[:])
            return c_new

        # ============================================ phase A
        with ExitStack() as actx:
            gpoolA = actx.enter_context(tc.tile_pool(name="gatesA", bufs=3))
            spoolA = actx.enter_context(tc.tile_pool(name="sgA", bufs=3))
            dpoolA = actx.enter_context(tc.tile_pool(name="dA", bufs=3))
            cpoolA = actx.enter_context(tc.tile_pool(name="cA", bufs=4))
            rpsumA = actx.enter_context(
                tc.tile_pool(name="rpsA", bufs=4, space="PSUM"))
            cellA = (gpoolA, spoolA, dpoolA, cpoolA)

            gath = actx.enter_context(tc.tile_pool(name="gath", bufs=3))
            tpsum = actx.enter_context(
                tc.tile_pool(name="tps", bufs=2, space="PSUM"))
            xppool = actx.enter_context(
                tc.tile_pool(name="xpps", bufs=2, space="PSUM"))
            fspans = actx.enter_context(tc.tile_pool(name="fspan", bufs=2))
            bspans = actx.enter_context(tc.tile_pool(name="bspan", bufs=2))
            p1stg = actx.enter_context(tc.tile_pool(name="p1stg", bufs=2))

            xt = const.tile([128, 2, T * BC], BF16)

            chunks_done = set()

            def produce_chunks(span):
                for g in range(4 * span, 4 * span + 4):
                    if g in chunks_done:
                        continue
                    chunks_done.add(g)
                    gt = gath.tile([128, E], F32, tag="graw")
                    nc.gpsimd.indirect_dma_start(
                        out=gt[:], out_offset=None, in_=emb_d.ap(),
                        in_offset=IndirectOffsetOnAxis(
                            ap=idx_sb[:, g:g + 1], axis=0))
                    bt = gath.tile([128, E], BF16, tag="gbf")
                    nc.vector.tensor_copy(bt[:], gt[:])
                    for k in range(2):
                        tp = tpsum.tile([128, 128], BF16)
                        nc.tensor.transpose(
                            tp[:], bt[:, 128 * k:128 * (k + 1)], ident[:])
                        nc.vector.tensor_copy(
                            xt[:, k, 128 * g:128 * (g + 1)], tp[:])

            def produce_span_l0(pool, wih, bias, s):
                span = pool.tile([128, SPC], BF16)
                sv = span[:].rearrange("p (t m u) -> p t m u", m=8, u=BC)
                for m in range(8):
                    ps = xppool.tile([128, 512], F32)
                    for k in range(2):
                        nc.tensor.matmul(
                            ps[:],
                            lhsT=wih[:, k, 128 * m:128 * (m + 1)],
                            rhs=xt[:, k, 512 * s:512 * (s + 1)],
                            start=(k == 0), stop=(k == 1))
                    with nc.allow_low_precision("bias-add into bf16 span"):
                        nc.vector.tensor_scalar_add(
                            sv[:, :, m, :],
                            ps[:].rearrange("p (t u) -> p t u", u=BC),
                            bias[:, m:m + 1])
                return span

            vyf = y0f[:].rearrange("p (t k u) -> p t k u", k=2, u=BC)

            def produce_p1f_span(blk):
                stg = p1stg.tile([128, SPC], BF16)
                svv = stg[:].rearrange("p (t m u) -> p t m u", m=8, u=BC)
                for m in range(8):
                    ps = xppool.tile([128, 512], F32)
                    for k in range(2):
                        nc.tensor.matmul(
                            ps[:],
                            lhsT=wsb["w_ih1f"][:, k, 128 * m:128 * (m + 1)],
                            rhs=vyf[:, 16 * blk:16 * (blk + 1), k, :],
                            start=(k == 0), stop=(k == 1))
                    with nc.allow_low_precision("bias-add into bf16 span"):
                        nc.vector.tensor_scalar_add(
                            svv[:, :, m, :],
                            ps[:].rearrange("p (t u) -> p t u", u=BC),
                            wsb["b1f"][:, m:m + 1])
                nc.sync.dma_start(
                    out=p1f_dram.ap()[:, SPC * blk:SPC * (blk + 1)],
                    in_=stg[:])

            produce_chunks(0)
            if SP > 1:
                produce_chunks(SP - 1)
            fcur = produce_span_l0(fspans, wsb["w_ih0f"], wsb["b0f"], 0)
            bcur = produce_span_l0(bspans, wsb["w_ih0b"], wsb["b0b"], SP - 1)

            cf = cb = None
            hf_prev = hb_prev = None
            for blk in range(SP):
                if blk + 1 < SP:
                    produce_chunks(blk + 1)
                    produce_chunks(SP - 2 - blk)
                    fnext = produce_span_l0(
                        fspans, wsb["w_ih0f"], wsb["b0f"], blk + 1)
                    bnext = produce_span_l0(
                        bspans, wsb["w_ih0b"], wsb["b0b"], SP - 2 - blk)
                for toff in range(16):
                    t = 16 * blk + toff
                    tp_ = T - 1 - t
                    # layer-0 forward, time t
                    fsl = fcur[:, GB * toff:GB * (toff + 1)]
                    ps = emit_recur(rpsumA, wsb["w_hh0f"], hf_prev, fsl) \
                        if t > 0 else None
                    h_out = y0f[:, 64 * t:64 * (t + 1)]
                    cf = emit_cell(cellA, ps, fsl, cf, h_out)
                    hf_prev = h_out
                    # layer-0 backward, time position tp_
                    bsl = bcur[:, GB * (15 - toff):GB * (16 - toff)]
                    ps = emit_recur(rpsumA, wsb["w_hh0b"], hb_prev, bsl) \
                        if t > 0 else None
                    h_out = y0b[:, 64 * tp_:64 * (tp_ + 1)]
                    cb = emit_cell(cellA, ps, bsl, cb, h_out)
                    hb_prev = h_out
                produce_p1f_span(blk)
                if blk + 1 < SP:
                    fcur, bcur = fnext, bnext

        # ============================================ phase B
        with ExitStack() as bctx:
            gpoolB = bctx.enter_context(tc.tile_pool(name="gatesB", bufs=3))
            spoolB = bctx.enter_context(tc.tile_pool(name="sgB", bufs=3))
            dpoolB = bctx.enter_context(tc.tile_pool(name="dB", bufs=3))
            cpoolB = bctx.enter_context(tc.tile_pool(name="cB", bufs=4))
            h1pool = bctx.enter_context(tc.tile_pool(name="h1", bufs=3))
            rpsumB = bctx.enter_context(
                tc.tile_pool(name="rpsB", bufs=3, space="PSUM"))
            bppsum = bctx.enter_context(
                tc.tile_pool(name="bpps", bufs=1, space="PSUM"))
            finpool = bctx.enter_context(tc.tile_pool(name="fin", bufs=2))
            mgpool = bctx.enter_context(tc.tile_pool(name="mg", bufs=2))
            cellB = (gpoolB, spoolB, dpoolB, cpoolB)

            vyb = y0b[:].rearrange("p (t k u) -> p t k u", k=2, u=BC)

            def produce_merged(blk):
                fin = finpool.tile([128, SPC], BF16)
                nc.sync.dma_start(
                    out=fin[:],
                    in_=p1f_dram.ap()[:, SPC * blk:SPC * (blk + 1)])
                mg = mgpool.tile([128, SPC], BF16)
                for half in range(2):
                    ps = bppsum.tile([128, 8, 8 * BC], F32)
                    hs0 = 16 * blk + 8 * half
                    for m in range(8):
                        for k in range(2):
                            nc.tensor.matmul(
                                ps[:, m, :],
                                lhsT=wsb["w_ih1f"][:, 2 + k,
                                                   128 * m:128 * (m + 1)],
                                rhs=vyb[:, hs0:hs0 + 8, k, :],
                                start=(k == 0), stop=(k == 1))
                    half_sl = slice(SPC // 2 * half, SPC // 2 * (half + 1))
                    with nc.allow_low_precision("xproj merge in bf16"):
                        nc.vector.tensor_add(
                            mg[:, half_sl].rearrange(
                                "p (t m u) -> p t m u", m=8, u=BC),
                            fin[:, half_sl].rearrange(
                                "p (t m u) -> p t m u", m=8, u=BC),
                            ps[:].rearrange("p m (t u) -> p t m u", u=BC))
                return mg

            mcur = produce_merged(0)
            c1 = None
            h1_prev = None
            for blk in range(SP):
                if blk + 1 < SP:
                    mnext = produce_merged(blk + 1)
                for toff in range(16):
                    t = 16 * blk + toff
                    msl = mcur[:, GB * toff:GB * (toff + 1)]
                    ps = emit_recur(rpsumB, wsb["w_hh1f"], h1_prev, msl) \
                        if t > 0 else None
                    if t == T - 1:
                        h_out = out_sb[:, 0:64]
                    else:
                        h1 = h1pool.tile([128, 64], BF16)
                        h_out = h1[:]
                    c1 = emit_cell(cellB, ps, msl, c1, h_out)
                    h1_prev = h_out
                if blk + 1 < SP:
                    mcur = mnext

            # single layer-1 backward step (output position T-1, zero state)
            ps = rpsumB.tile([128, GB], F32)
            for m in range(8):
                for k in range(4):
                    src = y0f if k < 2 else y0b
                    kk = k % 2
                    nc.tensor.matmul(
                        ps[:, BC * m:BC * (m + 1)],
                        lhsT=wsb["w_ih1b"][:, k, 128 * m:128 * (m + 1)],
                        rhs=src[:, 64 * (T - 1) + 32 * kk:
                                64 * (T - 1) + 32 * (kk + 1)],
                        start=(k == 0), stop=(k == 3))
            g1b = gpoolB.tile([128, GB], F32)
            for m in range(8):
                nc.scalar.activation(
                    g1b[:, BC * m:BC * (m + 1)], ps[:, BC * m:BC * (m + 1)],
                    AF.Identity, bias=wsb["b1b"][:, m:m + 1])
            emit_cell(cellB, None, g1b[:], None, out_sb[:, 64:128])

            nc.sync.dma_start(out=out_d.ap(), in_=out_sb[:])

    nc.compile()
    return nc


# ---------------------------------------------------------------- entry

_CACHE = {}
_last_res = None


def _get_program(T):
    if T not in _CACHE:
        _CACHE[T] = build_program(T)
    return _CACHE[T]


def kernel(**inputs):
    return _kernel_impl(inputs, T_FULL)


def _kernel_impl(inputs, T):
    idc = np.asarray(inputs["utterances_idc"]).astype(np.int32)
    emb = np.asarray(inputs["emb"], np.float32)
    w = _prep_weights(inputs)

    nc = _get_program(T)

    in_maps = []
    for c in range(N_CORES):
        idc_c = idc[c * BC:(c + 1) * BC, :T]
        m = {"idxr": _reorder_idx(idc_c, T), "emb": emb}
        m.update(w)
        in_maps.append(m)

    global _last_res
    res = run_bass_kernel_spmd(nc, in_maps, list(range(N_CORES)))
    _last_res = res

    outs = []
    for c in range(N_CORES):
        o = res.results[c]["out"]  # [128, 4*BC]: [h1f k0,k1 | h1b k0,k1] x u
        o4 = o.reshape(128, 4, BC)
        # h1f dims: k-tile major -> [256, BC]; same for h1b
        h1f = np.concatenate([o4[:, 0, :], o4[:, 1, :]], 0)  # [256, BC]
        h1b = np.concatenate([o4[:, 2, :], o4[:, 3, :]], 0)
        outs.append(np.concatenate([h1f.T, h1b.T], 1))  # [BC, 512]
    return np.concatenate(outs, 0).astype(np.float32)


# revision 13
# speedup vs baseline: 1.0358x; 1.0358x over previous
"""Trainium2 Bass kernel for a 2-layer bidirectional LSTM encoder.

Model (matches the reference):
  x = emb[idc]                      # [B=256, T=128, E=256]
  y0 = biLSTM_0(x)                  # H=256 per direction
  y1 = biLSTM_1(y0)
  out = y1[last timestep]           # [256, 512]

Sharding: data-parallel over the 256 utterances, 32 per NeuronCore, no
collectives.  Weights/embedding are replicated.  Structural shortcuts:
  - layer-1 backward only needs ONE step (output keeps position T-1, which is
    the first step of the reversed scan, from zero state).
  - layer-1 forward needs the full chain.

Per-core device program (gate-major layout: gate/hidden dims on partitions,
batch on the free axis, so no per-step transpose is needed):
  A) embedding gather (indirect DMA) -> cast bf16 -> PE transpose -> X.T;
     batched input projections for layer-0 f/b (PSUM -> +bias -> SBUF spans);
     128 interleaved steps of the l0f and l0b recurrences; h-seqs kept in
     SBUF (Y0f / Y0b); layer-1 fwd input projection (h0f part) batched into
     DRAM as it becomes available.
  B) layer-1 fwd chain: recurrent matmuls + (fpart-from-DRAM + bpart-batched)
     projections merged; then the single layer-1 bwd step; output staging.

Matmuls/weights/hidden in bf16 (fp32 PSUM accumulate); cell state c in fp32.
"""

import os
import sys

import numpy as np

for _p in ("/opt/trn_rl_repo",):
    if _p not in sys.path and os.path.isdir(_p):
        sys.path.insert(0, _p)

import ml_dtypes
from contextlib import ExitStack

import concourse.bacc as bacc
import concourse.bass as bass
import concourse.mybir as mybir
import concourse.tile as tile
from concourse.bass import IndirectOffsetOnAxis
from concourse.bass_utils import run_bass_kernel_spmd
from concourse.masks import make_identity

F32 = mybir.dt.float32
BF16 = mybir.dt.bfloat16
I32 = mybir.dt.int32
AF = mybir.ActivationFunctionType

V, E, H = 50257, 256, 256
NUM_UTT = 256
N_CORES = 8
BC = NUM_UTT // N_CORES  # 32 utterances per core
T_FULL = 128

bf16 = ml_dtypes.bfloat16


# ---------------------------------------------------------------- host prep

def _perm_rows(w):
    # PyTorch gate order i,f,g,o (blocks of H rows) -> i,f,o,g
    i, f, g, o = (w[k * H:(k + 1) * H] for k in range(4))
    return np.concatenate([i, f, o, g], 0)


def _prep_weights(inputs):
    """Transpose/permute weights on the host (layout only, no math)."""
    out = {}
    for key in ("w_ih0f", "w_hh0f", "w_ih0b", "w_hh0b",
                "w_ih1f", "w_hh1f", "w_ih1b"):
        w = np.asarray(inputs[key], np.float32)
        out[key] = np.ascontiguousarray(_perm_rows(w).T).astype(bf16)
    for key in ("b0f", "b0b", "b1f", "b1b"):
        b = np.asarray(inputs[key], np.float32)
        bp = _perm_rows(b.reshape(4 * H, 1)).reshape(4 * H)
        out[key] = np.ascontiguousarray(bp.reshape(8, 128).T).astype(np.float32)
    return out


def _reorder_idx(idc_c, T):
    # [BC, T] -> [128, CH]; gather chunk g covers timesteps [TPC*g, TPC*(g+1))
    # for all BC utterances, position p = (t - TPC*g)*BC + (u)
    CH = T * BC // 128
    a = np.ascontiguousarray(idc_c.T).reshape(CH, 128).T  # [128, CH]
    return np.ascontiguousarray(a).astype(np.int32)


# ---------------------------------------------------------------- device IR

def build_program(T):
    assert T % 16 == 0 and 128 % BC == 0
    CH = T * BC // 128       # gather chunks (4 timesteps each)
    SP = T // 16             # 16-step spans
    SPC = 16 * 8 * BC        # columns per span: (t, m, u) = 4096
    GB = 8 * BC              # gate columns per step = 256

    nc = bacc.Bacc("TRN2", target_bir_lowering=False, debug=False)

    idxr_d = nc.declare_dram_parameter("idxr", [128, CH], I32, isOutput=False)
    emb_d = nc.declare_dram_parameter("emb", [V, E], F32, isOutput=False)
    wd = {}
    for key, shape in (
        ("w_ih0f", [E, 1024]), ("w_hh0f", [H, 1024]),
        ("w_ih0b", [E, 1024]), ("w_hh0b", [H, 1024]),
        ("w_ih1f", [2 * H, 1024]), ("w_hh1f", [H, 1024]),
        ("w_ih1b", [2 * H, 1024]),
    ):
        wd[key] = nc.declare_dram_parameter(key, shape, BF16, isOutput=False)
    for key in ("b0f", "b0b", "b1f", "b1b"):
        wd[key] = nc.declare_dram_parameter(key, [128, 8], F32, isOutput=False)
    out_d = nc.declare_dram_parameter("out", [128, 4 * BC], F32, isOutput=True)

    p1f_dram = nc.dram_tensor("p1f_dram", [128, T * GB], BF16)

    with tile.TileContext(nc) as tc, ExitStack() as octx:
        const = octx.enter_context(tc.tile_pool(name="const", bufs=1))

        ident = const.tile([128, 128], BF16)
        make_identity(nc, ident[:])

        idx_sb = const.tile([128, CH], I32)
        nc.sync.dma_start(out=idx_sb[:], in_=idxr_d.ap())

        wsb = {}
        for key, kt in (("w_ih0f", 2), ("w_hh0f", 2), ("w_ih0b", 2),
                        ("w_hh0b", 2), ("w_ih1f", 4), ("w_hh1f", 2),
                        ("w_ih1b", 4)):
            wsb[key] = const.tile([128, kt, 1024], BF16, name=key, tag=key)
            nc.sync.dma_start(
                out=wsb[key][:],
                in_=wd[key].ap().rearrange("(k p) n -> p k n", p=128))
        for key in ("b0f", "b0b", "b1f", "b1b"):
            wsb[key] = const.tile([128, 8], F32, name=key, tag=key)
            nc.sync.dma_start(out=wsb[key][:], in_=wd[key].ap())

        # h sequences of layer 0, both dirs; cols = t*64 + k*32 + u
        y0f = const.tile([128, T * 64], BF16)
        y0b = const.tile([128, T * 64], BF16)
        out_sb = const.tile([128, 4 * BC], F32)

        # -------------------------------------------------- helpers
        def emit_recur(rpool, whh, h_prev, xsl):
            """gates PSUM = xsl (via identity matmul) + W_hh @ h_prev.

            The identity matmul has no dependency on h_prev, so it can be
            scheduled during the previous step's activation/cell tail.
            """
            ps = rpool.tile([128, GB], F32)
            nc.tensor.matmul(ps[:], lhsT=ident[:], rhs=xsl,
                             start=True, stop=False)
            for m in range(8):
                for k in range(2):
                    nc.tensor.matmul(
                        ps[:, BC * m:BC * (m + 1)],
                        lhsT=whh[:, k, 128 * m:128 * (m + 1)],
                        rhs=h_prev[:, 32 * k:32 * (k + 1)],
                        start=False, stop=(m == 7 and k == 1))
            return ps

        def emit_cell(pools, ps, xsl, c_prev, h_out):
            """One LSTM cell update in gate-major layout.

            ps: [128, GB] f32 PSUM gates (x-proj + recurrent), or None at
                step 0 (then xsl already is the full pre-activation)
            h_out: [128, 64] destination AP for the new hidden state
            returns the new cell state tile [128, 64] f32
            """
            gpool, spool, dpool, cpool = pools
            g_ap = xsl if ps is None else ps
            sg = spool.tile([128, 6 * BC], BF16, tag="sg")
            nc.scalar.activation(sg[:], g_ap[:, :6 * BC], AF.Sigmoid)
            tg = spool.tile([128, 2 * BC], BF16, tag="tg")
            nc.scalar.activation(tg[:], g_ap[:, 6 * BC:8 * BC], AF.Tanh)
            c_new = cpool.tile([128, 2 * BC], F32)
            if c_prev is None:
                t1 = dpool.tile([128, 2 * BC], BF16, tag="t1")
                nc.vector.tensor_mul(t1[:], sg[:, :2 * BC], tg[:])
                nc.vector.tensor_copy(c_new[:], t1[:])
            else:
                u = dpool.tile([128, 2 * BC], F32, tag="u")
                nc.vector.tensor_mul(u[:], sg[:, 2 * BC:4 * BC], c_prev[:])
                t1 = dpool.tile([128, 2 * BC], BF16, tag="t1")
                nc.vector.tensor_mul(t1[:], sg[:, :2 * BC], tg[:])
                nc.vector.tensor_add(c_new[:], u[:], t1[:])
            tc_ = dpool.tile([128, 2 * BC], BF16, tag="tc")
            nc.scalar.activation(tc_[:], c_new[:], AF.Tanh)
            nc.vector.tensor_mul(h_out, sg[:, 4 * BC:6 * BC], tc_[:])
            return c_new

        # ============================================ phase A
        with ExitStack() as actx:
            gpoolA = actx.enter_context(tc.tile_pool(name="gatesA", bufs=3))
            spoolA = actx.enter_context(tc.tile_pool(name="sgA", bufs=3))
            dpoolA = actx.enter_context(tc.tile_pool(name="dA", bufs=3))
            cpoolA = actx.enter_context(tc.tile_pool(name="cA", bufs=4))
            rpsumA = actx.enter_context(
                tc.tile_pool(name="rpsA", bufs=4, space="PSUM"))
            cellA = (gpoolA, spoolA, dpoolA, cpoolA)

            gath = actx.enter_context(tc.tile_pool(name="gath", bufs=3))
            tpsum = actx.enter_context(
                tc.tile_pool(name="tps", bufs=2, space="PSUM"))
            xppool = actx.enter_context(
                tc.tile_pool(name="xpps", bufs=2, space="PSUM"))
            fspans = actx.enter_context(tc.tile_pool(name="fspan", bufs=2))
            bspans = actx.enter_context(tc.tile_pool(name="bspan", bufs=2))
            p1stg = actx.enter_context(tc.tile_pool(name="p1stg", bufs=2))

            xt = const.tile([128, 2, T * BC], BF16)

            chunks_done = set()

            def produce_chunks(span):
                for g in range(4 * span, 4 * span + 4):
                    if g in chunks_done:
                        continue
                    chunks_done.add(g)
                    gt = gath.tile([128, E], F32, tag="graw")
                    nc.gpsimd.indirect_dma_start(
                        out=gt[:], out_offset=None, in_=emb_d.ap(),
                        in_offset=IndirectOffsetOnAxis(
                            ap=idx_sb[:, g:g + 1], axis=0))
                    bt = gath.tile([128, E], BF16, tag="gbf")
                    nc.vector.tensor_copy(bt[:], gt[:])
                    for k in range(2):
                        tp = tpsum.tile([128, 128], BF16)
                        nc.tensor.transpose(
                            tp[:], bt[:, 128 * k:128 * (k + 1)], ident[:])
                        nc.vector.tensor_copy(
                            xt[:, k, 128 * g:128 * (g + 1)], tp[:])

            def produce_span_l0(pool, wih, bias, s):
                span = pool.tile([128, SPC], BF16)
                sv = span[:].rearrange("p (t m u) -> p t m u", m=8, u=BC)
                for m in range(8):
                    ps = xppool.tile([128, 512], F32)
                    for k in range(2):
                        nc.tensor.matmul(
                            ps[:],
                            lhsT=wih[:, k, 128 * m:128 * (m + 1)],
                            rhs=xt[:, k, 512 * s:512 * (s + 1)],
                            start=(k == 0), stop=(k == 1))
                    nc.scalar.activation(
                        sv[:, :, m, :],
                        ps[:].rearrange("p (t u) -> p t u", u=BC),
                        AF.Identity, bias=bias[:, m:m + 1])
                return span

            vyf = y0f[:].rearrange("p (t k u) -> p t k u", k=2, u=BC)

            def produce_p1f_span(blk):
                stg = p1stg.tile([128, SPC], BF16)
                svv = stg[:].rearrange("p (t m u) -> p t m u", m=8, u=BC)
                for m in range(8):
                    ps = xppool.tile([128, 512], F32)
                    for k in range(2):
                        nc.tensor.matmul(
                            ps[:],
                            lhsT=wsb["w_ih1f"][:, k, 128 * m:128 * (m + 1)],
                            rhs=vyf[:, 16 * blk:16 * (blk + 1), k, :],
                            start=(k == 0), stop=(k == 1))
                    nc.scalar.activation(
                        svv[:, :, m, :],
                        ps[:].rearrange("p (t u) -> p t u", u=BC),
                        AF.Identity, bias=wsb["b1f"][:, m:m + 1])
                nc.sync.dma_start(
                    out=p1f_dram.ap()[:, SPC * blk:SPC * (blk + 1)],
                    in_=stg[:])

            produce_chunks(0)
            if SP > 1:
                produce_chunks(SP - 1)
            fcur = produce_span_l0(fspans, wsb["w_ih0f"], wsb["b0f"], 0)
            bcur = produce_span_l0(bspans, wsb["w_ih0b"], wsb["b0b"], SP - 1)

            cf = cb = None
            hf_prev = hb_prev = None
            for blk in range(SP):
                if blk + 1 < SP:
                    produce_chunks(blk + 1)
                    produce_chunks(SP - 2 - blk)
                    fnext = produce_span_l0(
                        fspans, wsb["w_ih0f"], wsb["b0f"], blk + 1)
                    bnext = produce_span_l0(
                        bspans, wsb["w_ih0b"], wsb["b0b"], SP - 2 - blk)
                for toff in range(16):
                    t = 16 * blk + toff
                    tp_ = T - 1 - t
                    # layer-0 forward, time t
                    fsl = fcur[:, GB * toff:GB * (toff + 1)]
                    ps = emit_recur(rpsumA, wsb["w_hh0f"], hf_prev, fsl) \
                        if t > 0 else None
                    h_out = y0f[:, 64 * t:64 * (t + 1)]
                    cf = emit_cell(cellA, ps, fsl, cf, h_out)
                    hf_prev = h_out
                    # layer-0 backward, time position tp_
                    bsl = bcur[:, GB * (15 - toff):GB * (16 - toff)]
                    ps = emit_recur(rpsumA, wsb["w_hh0b"], hb_prev, bsl) \
                        if t > 0 else None
                    h_out = y0b[:, 64 * tp_:64 * (tp_ + 1)]
                    cb = emit_cell(cellA, ps, bsl, cb, h_out)
                    hb_prev = h_out
                produce_p1f_span(blk)
                if blk + 1 < SP:
                    fcur, bcur = fnext, bnext

        # ============================================ phase B
        with ExitStack() as bctx:
            gpoolB = bctx.enter_context(tc.tile_pool(name="gatesB", bufs=3))
            spoolB = bctx.enter_context(tc.tile_pool(name="sgB", bufs=3))
            dpoolB = bctx.enter_context(tc.tile_pool(name="dB", bufs=3))
            cpoolB = bctx.enter_context(tc.tile_pool(name="cB", bufs=4))
            h1pool = bctx.enter_context(tc.tile_pool(name="h1", bufs=3))
            rpsumB = bctx.enter_context(
                tc.tile_pool(name="rpsB", bufs=3, space="PSUM"))
            bppsum = bctx.enter_context(
                tc.tile_pool(name="bpps", bufs=1, space="PSUM"))
            finpool = bctx.enter_context(tc.tile_pool(name="fin", bufs=2))
            mgpool = bctx.enter_context(tc.tile_pool(name="mg", bufs=2))
            cellB = (gpoolB, spoolB, dpoolB, cpoolB)

            vyb = y0b[:].rearrange("p (t k u) -> p t k u", k=2, u=BC)

            def produce_merged(blk):
                fin = finpool.tile([128, SPC], BF16)
                nc.sync.dma_start(
                    out=fin[:],
                    in_=p1f_dram.ap()[:, SPC * blk:SPC * (blk + 1)])
                mg = mgpool.tile([128, SPC], BF16)
                for half in range(2):
                    ps = bppsum.tile([128, 8, 8 * BC], F32)
                    hs0 = 16 * blk + 8 * half
                    for m in range(8):
                        for k in range(2):
                            nc.tensor.matmul(
                                ps[:, m, :],
                                lhsT=wsb["w_ih1f"][:, 2 + k,
                                                   128 * m:128 * (m + 1)],
                                rhs=vyb[:, hs0:hs0 + 8, k, :],
                                start=(k == 0), stop=(k == 1))
                    half_sl = slice(SPC // 2 * half, SPC // 2 * (half + 1))
                    with nc.allow_low_precision("xproj merge in bf16"):
                        nc.vector.tensor_add(
                            mg[:, half_sl].rearrange(
                                "p (t m u) -> p t m u", m=8, u=BC),
                            fin[:, half_sl].rearrange(
                                "p (t m u) -> p t m u", m=8, u=BC),
                            ps[:].rearrange("p m (t u) -> p t m u", u=BC))
                return mg

            mcur = produce_merged(0)
            c1 = None
            h1_prev = None
            for blk in range(SP):
                if blk + 1 < SP:
                    mnext = produce_merged(blk + 1)
                for toff in range(16):
                    t = 16 * blk + toff
                    msl = mcur[:, GB * toff:GB * (toff + 1)]
                    ps = emit_recur(rpsumB, wsb["w_hh1f"], h1_prev, msl) \
                        if t > 0 else None
                    if t == T - 1:
                        h_out = out_sb[:, 0:64]
                    else:
                        h1 = h1pool.tile([128, 64], BF16)
                        h_out = h1[:]
                    c1 = emit_cell(cellB, ps, msl, c1, h_out)
                    h1_prev = h_out
                if blk + 1 < SP:
                    mcur = mnext

            # single layer-1 backward step (output position T-1, zero state)
            ps = rpsumB.tile([128, GB], F32)
            for m in range(8):
                for k in range(4):
                    src = y0f if k < 2 else y0b
                    kk = k % 2
                    nc.tensor.matmul(
                        ps[:, BC * m:BC * (m + 1)],
                        lhsT=wsb["w_ih1b"][:, k, 128 * m:128 * (m + 1)],
                        rhs=src[:, 64 * (T - 1) + 32 * kk:
                                64 * (T - 1) + 32 * (kk + 1)],
                        start=(k == 0), stop=(k == 3))
            g1b = gpoolB.tile([128, GB], F32)
            for m in range(8):
                nc.scalar.activation(
                    g1b[:, BC * m:BC * (m + 1)], ps[:, BC * m:BC * (m + 1)],
                    AF.Identity, bias=wsb["b1b"][:, m:m + 1])
            emit_cell(cellB, None, g1b[:], None, out_sb[:, 64:128])

            nc.sync.dma_start(out=out_d.ap(), in_=out_sb[:])

    nc.compile()
    return nc


# ---------------------------------------------------------------- entry

_CACHE = {}
_last_res = None


def _get_program(T):
    if T not in _CACHE:
        _CACHE[T] = build_program(T)
    return _CACHE[T]


def kernel(**inputs):
    return _kernel_impl(inputs, T_FULL)


def _kernel_impl(inputs, T):
    idc = np.asarray(inputs["utterances_idc"]).astype(np.int32)
    emb = np.asarray(inputs["emb"], np.float32)
    w = _prep_weights(inputs)

    nc = _get_program(T)

    in_maps = []
    for c in range(N_CORES):
        idc_c = idc[c * BC:(c + 1) * BC, :T]
        m = {"idxr": _reorder_idx(idc_c, T), "emb": emb}
        m.update(w)
        in_maps.append(m)

    global _last_res
    res = run_bass_kernel_spmd(nc, in_maps, list(range(N_CORES)))
    _last_res = res

    outs = []
    for c in range(N_CORES):
        o = res.results[c]["out"]  # [128, 4*BC]: [h1f k0,k1 | h1b k0,k1] x u
        o4 = o.reshape(128, 4, BC)
        # h1f dims: k-tile major -> [256, BC]; same for h1b
        h1f = np.concatenate([o4[:, 0, :], o4[:, 1, :]], 0)  # [256, BC]
        h1b = np.concatenate([o4[:, 2, :], o4[:, 3, :]], 0)
        outs.append(np.concatenate([h1f.T, h1b.T], 1))  # [BC, 512]
    return np.concatenate(outs, 0).astype(np.float32)


# revision 14
# speedup vs baseline: 1.0905x; 1.0528x over previous
"""Trainium2 Bass kernel for a 2-layer bidirectional LSTM encoder.

Model (matches the reference):
  x = emb[idc]                      # [B=256, T=128, E=256]
  y0 = biLSTM_0(x)                  # H=256 per direction
  y1 = biLSTM_1(y0)
  out = y1[last timestep]           # [256, 512]

Sharding: data-parallel over the 256 utterances, 32 per NeuronCore, no
collectives.  Weights/embedding are replicated.  Structural shortcuts:
  - layer-1 backward only needs ONE step (output keeps position T-1, which is
    the first step of the reversed scan, from zero state).
  - layer-1 forward needs the full chain.

Per-core device program (gate-major layout: gate/hidden dims on partitions,
batch on the free axis, so no per-step transpose is needed):
  A) embedding gather (indirect DMA) -> cast bf16 -> PE transpose -> X.T;
     batched input projections for layer-0 f/b (PSUM -> +bias -> SBUF spans);
     128 interleaved steps of the l0f and l0b recurrences; h-seqs kept in
     SBUF (Y0f / Y0b); layer-1 fwd input projection (h0f part) batched into
     DRAM as it becomes available.
  B) layer-1 fwd chain: recurrent matmuls + (fpart-from-DRAM + bpart-batched)
     projections merged; then the single layer-1 bwd step; output staging.

Matmuls/weights/hidden in bf16 (fp32 PSUM accumulate); cell state c in fp32.
"""

import os
import sys

import numpy as np

for _p in ("/opt/trn_rl_repo",):
    if _p not in sys.path and os.path.isdir(_p):
        sys.path.insert(0, _p)

import ml_dtypes
from contextlib import ExitStack

import concourse.bacc as bacc
import concourse.bass as bass
import concourse.mybir as mybir
import concourse.tile as tile
from concourse.bass import IndirectOffsetOnAxis
from concourse.bass_utils import run_bass_kernel_spmd
from concourse.masks import make_identity

F32 = mybir.dt.float32
BF16 = mybir.dt.bfloat16
I32 = mybir.dt.int32
AF = mybir.ActivationFunctionType

V, E, H = 50257, 256, 256
NUM_UTT = 256
N_CORES = 8
BC = NUM_UTT // N_CORES  # 32 utterances per core
T_FULL = 128

bf16 = ml_dtypes.bfloat16


# ---------------------------------------------------------------- host prep

def _perm_rows(w):
    # PyTorch gate order i,f,g,o (blocks of H rows) -> i,f,o,g
    i, f, g, o = (w[k * H:(k + 1) * H] for k in range(4))
    return np.concatenate([i, f, o, g], 0)


def _prep_weights(inputs):
    """Transpose/permute weights on the host (layout only, no math)."""
    out = {}
    for key in ("w_ih0f", "w_hh0f", "w_ih0b", "w_hh0b",
                "w_ih1f", "w_hh1f", "w_ih1b"):
        w = np.asarray(inputs[key], np.float32)
        out[key] = np.ascontiguousarray(_perm_rows(w).T).astype(bf16)
    for key in ("b0f", "b0b", "b1f", "b1b"):
        b = np.asarray(inputs[key], np.float32)
        bp = _perm_rows(b.reshape(4 * H, 1)).reshape(4 * H)
        out[key] = np.ascontiguousarray(bp.reshape(8, 128).T).astype(np.float32)
    return out


def _reorder_idx(idc_c, T):
    # [BC, T] -> [128, CH]; gather chunk g covers timesteps [TPC*g, TPC*(g+1))
    # for all BC utterances, position p = (t - TPC*g)*BC + (u)
    CH = T * BC // 128
    a = np.ascontiguousarray(idc_c.T).reshape(CH, 128).T  # [128, CH]
    return np.ascontiguousarray(a).astype(np.int32)


# ---------------------------------------------------------------- device IR

def build_program(T):
    assert T % 16 == 0 and 128 % BC == 0
    CH = T * BC // 128       # gather chunks (4 timesteps each)
    SP = T // 16             # 16-step spans
    SPC = 16 * 8 * BC        # columns per span: (t, m, u) = 4096
    GB = 8 * BC              # gate columns per step = 256

    nc = bacc.Bacc("TRN2", target_bir_lowering=False, debug=False)

    idxr_d = nc.declare_dram_parameter("idxr", [128, CH], I32, isOutput=False)
    emb_d = nc.declare_dram_parameter("emb", [V, E], F32, isOutput=False)
    wd = {}
    for key, shape in (
        ("w_ih0f", [E, 1024]), ("w_hh0f", [H, 1024]),
        ("w_ih0b", [E, 1024]), ("w_hh0b", [H, 1024]),
        ("w_ih1f", [2 * H, 1024]), ("w_hh1f", [H, 1024]),
        ("w_ih1b", [2 * H, 1024]),
    ):
        wd[key] = nc.declare_dram_parameter(key, shape, BF16, isOutput=False)
    for key in ("b0f", "b0b", "b1f", "b1b"):
        wd[key] = nc.declare_dram_parameter(key, [128, 8], F32, isOutput=False)
    out_d = nc.declare_dram_parameter("out", [128, 4 * BC], F32, isOutput=True)

    p1f_dram = nc.dram_tensor("p1f_dram", [128, T * GB], BF16)

    with tile.TileContext(nc) as tc, ExitStack() as octx:
        const = octx.enter_context(tc.tile_pool(name="const", bufs=1))

        ident = const.tile([128, 128], BF16)
        make_identity(nc, ident[:])

        idx_sb = const.tile([128, CH], I32)
        nc.sync.dma_start(out=idx_sb[:], in_=idxr_d.ap())

        wsb = {}
        for key, kt in (("w_ih0f", 2), ("w_hh0f", 2), ("w_ih0b", 2),
                        ("w_hh0b", 2), ("w_ih1f", 4), ("w_hh1f", 2),
                        ("w_ih1b", 4)):
            wsb[key] = const.tile([128, kt, 1024], BF16, name=key, tag=key)
            nc.sync.dma_start(
                out=wsb[key][:],
                in_=wd[key].ap().rearrange("(k p) n -> p k n", p=128))
        for key in ("b0f", "b0b", "b1f", "b1b"):
            wsb[key] = const.tile([128, 8], F32, name=key, tag=key)
            nc.sync.dma_start(out=wsb[key][:], in_=wd[key].ap())

        # h sequences of layer 0, both dirs; cols = t*64 + k*32 + u
        y0f = const.tile([128, T * 64], BF16)
        y0b = const.tile([128, T * 64], BF16)
        out_sb = const.tile([128, 4 * BC], F32)

        # -------------------------------------------------- helpers
        def emit_recur(rpool, whh, h_prev, xsl):
            ps = rpool.tile([128, GB], F32)
            for m in range(8):
                for k in range(2):
                    nc.tensor.matmul(
                        ps[:, BC * m:BC * (m + 1)],
                        lhsT=whh[:, k, 128 * m:128 * (m + 1)],
                        rhs=h_prev[:, 32 * k:32 * (k + 1)],
                        start=(k == 0), stop=(k == 1))
            return ps

        def emit_cell(pools, ps, xsl, c_prev, h_out):
            """One LSTM cell update in gate-major layout.

            ps: [128, GB] f32 PSUM recurrent gates, or None at step 0
            xsl: [128, GB] bf16 input-projection slice (includes bias)
            h_out: [128, 64] destination AP for the new hidden state
            returns the new cell state tile [128, 64] f32
            """
            gpool, spool, dpool, cpool = pools
            if ps is None:
                g_ap = xsl
            else:
                gates = gpool.tile([128, GB], F32)
                nc.vector.tensor_add(gates[:, :6 * BC], ps[:, :6 * BC],
                                     xsl[:, :6 * BC])
                nc.vector.tensor_add(gates[:, 6 * BC:], ps[:, 6 * BC:],
                                     xsl[:, 6 * BC:])
                g_ap = gates
            sg = spool.tile([128, 6 * BC], BF16, tag="sg")
            nc.scalar.activation(sg[:], g_ap[:, :6 * BC], AF.Sigmoid)
            tg = spool.tile([128, 2 * BC], BF16, tag="tg")
            nc.scalar.activation(tg[:], g_ap[:, 6 * BC:8 * BC], AF.Tanh)
            c_new = cpool.tile([128, 2 * BC], F32)
            if c_prev is None:
                t1 = dpool.tile([128, 2 * BC], BF16, tag="t1")
                nc.vector.tensor_mul(t1[:], sg[:, :2 * BC], tg[:])
                nc.vector.tensor_copy(c_new[:], t1[:])
            else:
                u = dpool.tile([128, 2 * BC], F32, tag="u")
                nc.vector.tensor_mul(u[:], sg[:, 2 * BC:4 * BC], c_prev[:])
                t1 = dpool.tile([128, 2 * BC], BF16, tag="t1")
                nc.vector.tensor_mul(t1[:], sg[:, :2 * BC], tg[:])
                nc.vector.tensor_add(c_new[:], u[:], t1[:])
            tc_ = dpool.tile([128, 2 * BC], BF16, tag="tc")
            nc.scalar.activation(tc_[:], c_new[:], AF.Tanh)
            nc.vector.tensor_mul(h_out, sg[:, 4 * BC:6 * BC], tc_[:])
            return c_new

        # ============================================ phase A
        with ExitStack() as actx:
            gpoolA = actx.enter_context(tc.tile_pool(name="gatesA", bufs=3))
            spoolA = actx.enter_context(tc.tile_pool(name="sgA", bufs=3))
            dpoolA = actx.enter_context(tc.tile_pool(name="dA", bufs=3))
            cpoolA = actx.enter_context(tc.tile_pool(name="cA", bufs=4))
            rpsumA = actx.enter_context(
                tc.tile_pool(name="rpsA", bufs=4, space="PSUM"))
            cellA = (gpoolA, spoolA, dpoolA, cpoolA)

            gath = actx.enter_context(tc.tile_pool(name="gath", bufs=3))
            tpsum = actx.enter_context(
                tc.tile_pool(name="tps", bufs=2, space="PSUM"))
            xppool = actx.enter_context(
                tc.tile_pool(name="xpps", bufs=2, space="PSUM"))
            fspans = actx.enter_context(tc.tile_pool(name="fspan", bufs=2))
            bspans = actx.enter_context(tc.tile_pool(name="bspan", bufs=2))
            p1stg = actx.enter_context(tc.tile_pool(name="p1stg", bufs=2))

            xt = const.tile([128, 2, T * BC], BF16)

            chunks_done = set()

            def produce_chunks(span):
                for g in range(4 * span, 4 * span + 4):
                    if g in chunks_done:
                        continue
                    chunks_done.add(g)
                    gt = gath.tile([128, E], F32, tag="graw")
                    nc.gpsimd.indirect_dma_start(
                        out=gt[:], out_offset=None, in_=emb_d.ap(),
                        in_offset=IndirectOffsetOnAxis(
                            ap=idx_sb[:, g:g + 1], axis=0))
                    bt = gath.tile([128, E], BF16, tag="gbf")
                    nc.vector.tensor_copy(bt[:], gt[:])
                    for k in range(2):
                        tp = tpsum.tile([128, 128], BF16)
                        nc.tensor.transpose(
                            tp[:], bt[:, 128 * k:128 * (k + 1)], ident[:])
                        nc.vector.tensor_copy(
                            xt[:, k, 128 * g:128 * (g + 1)], tp[:])

            def produce_span_l0(pool, wih, bias, s):
                span = pool.tile([128, SPC], BF16)
                sv = span[:].rearrange("p (t m u) -> p t m u", m=8, u=BC)
                for m in range(8):
                    ps = xppool.tile([128, 512], F32)
                    for k in range(2):
                        nc.tensor.matmul(
                            ps[:],
                            lhsT=wih[:, k, 128 * m:128 * (m + 1)],
                            rhs=xt[:, k, 512 * s:512 * (s + 1)],
                            start=(k == 0), stop=(k == 1))
                    nc.scalar.activation(
                        sv[:, :, m, :],
                        ps[:].rearrange("p (t u) -> p t u", u=BC),
                        AF.Identity, bias=bias[:, m:m + 1])
                return span

            vyf = y0f[:].rearrange("p (t k u) -> p t k u", k=2, u=BC)

            def produce_p1f_span(blk):
                stg = p1stg.tile([128, SPC], BF16)
                svv = stg[:].rearrange("p (t m u) -> p t m u", m=8, u=BC)
                for m in range(8):
                    ps = xppool.tile([128, 512], F32)
                    for k in range(2):
                        nc.tensor.matmul(
                            ps[:],
                            lhsT=wsb["w_ih1f"][:, k, 128 * m:128 * (m + 1)],
                            rhs=vyf[:, 16 * blk:16 * (blk + 1), k, :],
                            start=(k == 0), stop=(k == 1))
                    nc.scalar.activation(
                        svv[:, :, m, :],
                        ps[:].rearrange("p (t u) -> p t u", u=BC),
                        AF.Identity, bias=wsb["b1f"][:, m:m + 1])
                nc.sync.dma_start(
                    out=p1f_dram.ap()[:, SPC * blk:SPC * (blk + 1)],
                    in_=stg[:])

            produce_chunks(0)
            if SP > 1:
                produce_chunks(SP - 1)
            fcur = produce_span_l0(fspans, wsb["w_ih0f"], wsb["b0f"], 0)
            bcur = produce_span_l0(bspans, wsb["w_ih0b"], wsb["b0b"], SP - 1)

            cf = cb = None
            hf_prev = hb_prev = None
            for blk in range(SP):
                if blk + 1 < SP:
                    produce_chunks(blk + 1)
                    produce_chunks(SP - 2 - blk)
                    fnext = produce_span_l0(
                        fspans, wsb["w_ih0f"], wsb["b0f"], blk + 1)
                    bnext = produce_span_l0(
                        bspans, wsb["w_ih0b"], wsb["b0b"], SP - 2 - blk)
                for toff in range(16):
                    t = 16 * blk + toff
                    tp_ = T - 1 - t
                    # layer-0 forward, time t
                    fsl = fcur[:, GB * toff:GB * (toff + 1)]
                    ps = emit_recur(rpsumA, wsb["w_hh0f"], hf_prev, fsl) \
                        if t > 0 else None
                    h_out = y0f[:, 64 * t:64 * (t + 1)]
                    cf = emit_cell(cellA, ps, fsl, cf, h_out)
                    hf_prev = h_out
                    # layer-0 backward, time position tp_
                    bsl = bcur[:, GB * (15 - toff):GB * (16 - toff)]
                    ps = emit_recur(rpsumA, wsb["w_hh0b"], hb_prev, bsl) \
                        if t > 0 else None
                    h_out = y0b[:, 64 * tp_:64 * (tp_ + 1)]
                    cb = emit_cell(cellA, ps, bsl, cb, h_out)
                    hb_prev = h_out
                produce_p1f_span(blk)
                if blk + 1 < SP:
                    fcur, bcur = fnext, bnext

        # ============================================ phase B
        with ExitStack() as bctx:
            gpoolB = bctx.enter_context(tc.tile_pool(name="gatesB", bufs=3))
            spoolB = bctx.enter_context(tc.tile_pool(name="sgB", bufs=3))
            dpoolB = bctx.enter_context(tc.tile_pool(name="dB", bufs=3))
            cpoolB = bctx.enter_context(tc.tile_pool(name="cB", bufs=4))
            h1pool = bctx.enter_context(tc.tile_pool(name="h1", bufs=3))
            rpsumB = bctx.enter_context(
                tc.tile_pool(name="rpsB", bufs=3, space="PSUM"))
            bppsum = bctx.enter_context(
                tc.tile_pool(name="bpps", bufs=1, space="PSUM"))
            finpool = bctx.enter_context(tc.tile_pool(name="fin", bufs=2))
            mgpool = bctx.enter_context(tc.tile_pool(name="mg", bufs=2))
            cellB = (gpoolB, spoolB, dpoolB, cpoolB)

            vyb = y0b[:].rearrange("p (t k u) -> p t k u", k=2, u=BC)

            def produce_merged(blk):
                fin = finpool.tile([128, SPC], BF16)
                nc.sync.dma_start(
                    out=fin[:],
                    in_=p1f_dram.ap()[:, SPC * blk:SPC * (blk + 1)])
                mg = mgpool.tile([128, SPC], BF16)
                for half in range(2):
                    ps = bppsum.tile([128, 8, 8 * BC], F32)
                    hs0 = 16 * blk + 8 * half
                    for m in range(8):
                        for k in range(2):
                            nc.tensor.matmul(
                                ps[:, m, :],
                                lhsT=wsb["w_ih1f"][:, 2 + k,
                                                   128 * m:128 * (m + 1)],
                                rhs=vyb[:, hs0:hs0 + 8, k, :],
                                start=(k == 0), stop=(k == 1))
                    half_sl = slice(SPC // 2 * half, SPC // 2 * (half + 1))
                    with nc.allow_low_precision("xproj merge in bf16"):
                        nc.vector.tensor_add(
                            mg[:, half_sl].rearrange(
                                "p (t m u) -> p t m u", m=8, u=BC),
                            fin[:, half_sl].rearrange(
                                "p (t m u) -> p t m u", m=8, u=BC),
                            ps[:].rearrange("p m (t u) -> p t m u", u=BC))
                return mg

            mcur = produce_merged(0)
            c1 = None
            h1_prev = None
            for blk in range(SP):
                if blk + 1 < SP:
                    mnext = produce_merged(blk + 1)
                for toff in range(16):
                    t = 16 * blk + toff
                    msl = mcur[:, GB * toff:GB * (toff + 1)]
                    ps = emit_recur(rpsumB, wsb["w_hh1f"], h1_prev, msl) \
                        if t > 0 else None
                    if t == T - 1:
                        h_out = out_sb[:, 0:64]
                    else:
                        h1 = h1pool.tile([128, 64], BF16)
                        h_out = h1[:]
                    c1 = emit_cell(cellB, ps, msl, c1, h_out)
                    h1_prev = h_out
                if blk + 1 < SP:
                    mcur = mnext

            # single layer-1 backward step (output position T-1, zero state)
            ps = rpsumB.tile([128, GB], F32)
            for m in range(8):
                for k in range(4):
                    src = y0f if k < 2 else y0b
                    kk = k % 2
                    nc.tensor.matmul(
                        ps[:, BC * m:BC * (m + 1)],
                        lhsT=wsb["w_ih1b"][:, k, 128 * m:128 * (m + 1)],
                        rhs=src[:, 64 * (T - 1) + 32 * kk:
                                64 * (T - 1) + 32 * (kk + 1)],
                        start=(k == 0), stop=(k == 3))
            g1b = gpoolB.tile([128, GB], F32)
            for m in range(8):
                nc.scalar.activation(
                    g1b[:, BC * m:BC * (m + 1)], ps[:, BC * m:BC * (m + 1)],
                    AF.Identity, bias=wsb["b1b"][:, m:m + 1])
            emit_cell(cellB, None, g1b[:], None, out_sb[:, 64:128])

            nc.sync.dma_start(out=out_d.ap(), in_=out_sb[:])

    nc.compile()
    return nc


# ---------------------------------------------------------------- entry

_CACHE = {}
_last_res = None


def _get_program(T):
    if T not in _CACHE:
        _CACHE[T] = build_program(T)
    return _CACHE[T]


def kernel(**inputs):
    return _kernel_impl(inputs, T_FULL)


def _kernel_impl(inputs, T):
    idc = np.asarray(inputs["utterances_idc"]).astype(np.int32)
    emb = np.asarray(inputs["emb"], np.float32)
    w = _prep_weights(inputs)

    nc = _get_program(T)

    in_maps = []
    for c in range(N_CORES):
        idc_c = idc[c * BC:(c + 1) * BC, :T]
        m = {"idxr": _reorder_idx(idc_c, T), "emb": emb}
        m.update(w)
        in_maps.append(m)

    global _last_res
    res = run_bass_kernel_spmd(nc, in_maps, list(range(N_CORES)))
    _last_res = res

    outs = []
    for c in range(N_CORES):
        o = res.results[c]["out"]  # [128, 4*BC]: [h1f k0,k1 | h1b k0,k1] x u
        o4 = o.reshape(128, 4, BC)
        # h1f dims: k-tile major -> [256, BC]; same for h1b
        h1f = np.concatenate([o4[:, 0, :], o4[:, 1, :]], 0)  # [256, BC]
        h1b = np.concatenate([o4[:, 2, :], o4[:, 3, :]], 0)
        outs.append(np.concatenate([h1f.T, h1b.T], 1))  # [BC, 512]
    return np.concatenate(outs, 0).astype(np.float32)


# revision 20
# speedup vs baseline: 1.1868x; 1.0883x over previous
"""Trainium2 Bass kernel for a 2-layer bidirectional LSTM encoder.

Model (matches the reference):
  x = emb[idc]                      # [B=256, T=128, E=256]
  y0 = biLSTM_0(x)                  # H=256 per direction
  y1 = biLSTM_1(y0)
  out = y1[last timestep]           # [256, 512]

Sharding: data-parallel over the 256 utterances, 32 per NeuronCore, no
collectives.  Weights/embedding are replicated.  Structural shortcuts:
  - layer-1 backward only needs ONE step (output keeps position T-1, which is
    the first step of the reversed scan, from zero state).
  - layer-1 forward needs the full chain.

Per-core device program (gate-major layout: gate/hidden dims on partitions,
batch on the free axis, so no per-step transpose is needed):
  A) embedding gather (indirect DMA) -> cast bf16 -> PE transpose -> X.T;
     batched input projections for layer-0 f/b (PSUM -> +bias -> SBUF spans);
     128 interleaved steps of the l0f and l0b recurrences; h-seqs kept in
     SBUF (Y0f / Y0b); layer-1 fwd input projection (h0f part) batched into
     DRAM as it becomes available.
  B) layer-1 fwd chain: recurrent matmuls + (fpart-from-DRAM + bpart-batched)
     projections merged; then the single layer-1 bwd step; output staging.

Matmuls/weights/hidden in bf16 (fp32 PSUM accumulate); cell state c in fp32.
"""

import os
import sys

import numpy as np

for _p in ("/opt/trn_rl_repo",):
    if _p not in sys.path and os.path.isdir(_p):
        sys.path.insert(0, _p)

import ml_dtypes
from contextlib import ExitStack

import concourse.bacc as bacc
import concourse.bass as bass
import concourse.mybir as mybir
import concourse.tile as tile
from concourse.bass import IndirectOffsetOnAxis
from concourse.bass_utils import run_bass_kernel_spmd
from concourse.masks import make_identity

F32 = mybir.dt.float32
BF16 = mybir.dt.bfloat16
I32 = mybir.dt.int32
AF = mybir.ActivationFunctionType

V, E, H = 50257, 256, 256
NUM_UTT = 256
N_CORES = 8
BC = NUM_UTT // N_CORES  # 32 utterances per core
T_FULL = 128

bf16 = ml_dtypes.bfloat16


# ---------------------------------------------------------------- host prep

def _perm_rows(w):
    # PyTorch gate order i,f,g,o (blocks of H rows) -> i,f,o,g
    i, f, g, o = (w[k * H:(k + 1) * H] for k in range(4))
    return np.concatenate([i, f, o, g], 0)


def _prep_weights(inputs):
    """Transpose/permute weights on the host (layout only, no math)."""
    out = {}
    for key in ("w_ih0f", "w_hh0f", "w_ih0b", "w_hh0b",
                "w_ih1f", "w_hh1f", "w_ih1b"):
        w = np.asarray(inputs[key], np.float32)
        out[key] = np.ascontiguousarray(_perm_rows(w).T).astype(bf16)
    for key in ("b0f", "b0b", "b1f", "b1b"):
        b = np.asarray(inputs[key], np.float32)
        bp = _perm_rows(b.reshape(4 * H, 1)).reshape(4 * H)
        out[key] = np.ascontiguousarray(bp.reshape(8, 128).T).astype(np.float32)
    return out


def _reorder_idx(idc_c, T):
    # [BC, T] -> [128, CH]; gather chunk g covers timesteps [TPC*g, TPC*(g+1))
    # for all BC utterances, position p = (t - TPC*g)*BC + (u)
    CH = T * BC // 128
    a = np.ascontiguousarray(idc_c.T).reshape(CH, 128).T  # [128, CH]
    return np.ascontiguousarray(a).astype(np.int32)


# ---------------------------------------------------------------- device IR

def build_program(T):
    assert T % 16 == 0 and 128 % BC == 0
    CH = T * BC // 128       # gather chunks (4 timesteps each)
    SP = T // 16             # 16-step spans
    SPC = 16 * 8 * BC        # columns per span: (t, m, u) = 4096
    GB = 8 * BC              # gate columns per step = 256

    nc = bacc.Bacc("TRN2", target_bir_lowering=False, debug=False)

    idxr_d = nc.declare_dram_parameter("idxr", [128, CH], I32, isOutput=False)
    emb_d = nc.declare_dram_parameter("emb", [V, E], F32, isOutput=False)
    wd = {}
    for key, shape in (
        ("w_ih0f", [E, 1024]), ("w_hh0f", [H, 1024]),
        ("w_ih0b", [E, 1024]), ("w_hh0b", [H, 1024]),
        ("w_ih1f", [2 * H, 1024]), ("w_hh1f", [H, 1024]),
        ("w_ih1b", [2 * H, 1024]),
    ):
        wd[key] = nc.declare_dram_parameter(key, shape, BF16, isOutput=False)
    for key in ("b0f", "b0b", "b1f", "b1b"):
        wd[key] = nc.declare_dram_parameter(key, [128, 8], F32, isOutput=False)
    out_d = nc.declare_dram_parameter("out", [128, 4 * BC], F32, isOutput=True)

    p1f_dram = nc.dram_tensor("p1f_dram", [128, T * GB], BF16)

    with tile.TileContext(nc) as tc, ExitStack() as octx:
        const = octx.enter_context(tc.tile_pool(name="const", bufs=1))

        ident = const.tile([128, 128], BF16)
        make_identity(nc, ident[:])

        idx_sb = const.tile([128, CH], I32)
        nc.sync.dma_start(out=idx_sb[:], in_=idxr_d.ap())

        wsb = {}
        for key, kt in (("w_ih0f", 2), ("w_hh0f", 2), ("w_ih0b", 2),
                        ("w_hh0b", 2), ("w_ih1f", 4), ("w_hh1f", 2),
                        ("w_ih1b", 4)):
            wsb[key] = const.tile([128, kt, 1024], BF16, name=key, tag=key)
            nc.sync.dma_start(
                out=wsb[key][:],
                in_=wd[key].ap().rearrange("(k p) n -> p k n", p=128))
        for key in ("b0f", "b0b", "b1f", "b1b"):
            wsb[key] = const.tile([128, 8], F32, name=key, tag=key)
            nc.sync.dma_start(out=wsb[key][:], in_=wd[key].ap())

        # h sequences of layer 0, both dirs; cols = t*64 + k*32 + u
        y0f = const.tile([128, T * 64], BF16)
        y0b = const.tile([128, T * 64], BF16)
        out_sb = const.tile([128, 4 * BC], F32)

        # -------------------------------------------------- helpers
        def emit_recur(rpool, whh, h_prev, xsl):
            # Split the gates across two PSUM banks: i,f,o in one tile and
            # g in another, so the sigmoid path can start as soon as the 12
            # i/f/o matmuls finish while the 4 g matmuls + tanh overlap it.
            ps_ifo = rpool.tile([128, 6 * BC], F32, tag="ps_ifo")
            ps_g = rpool.tile([128, 2 * BC], F32, tag="ps_g")
            for m in range(6):
                for k in range(2):
                    nc.tensor.matmul(
                        ps_ifo[:, BC * m:BC * (m + 1)],
                        lhsT=whh[:, k, 128 * m:128 * (m + 1)],
                        rhs=h_prev[:, 32 * k:32 * (k + 1)],
                        start=(k == 0), stop=(k == 1))
            for m in range(6, 8):
                for k in range(2):
                    nc.tensor.matmul(
                        ps_g[:, BC * (m - 6):BC * (m - 5)],
                        lhsT=whh[:, k, 128 * m:128 * (m + 1)],
                        rhs=h_prev[:, 32 * k:32 * (k + 1)],
                        start=(k == 0), stop=(k == 1))
            return (ps_ifo, ps_g)

        def emit_cell(pools, ps, xsl, c_prev, h_out):
            """One LSTM cell update in gate-major layout.

            ps: (ps_ifo, ps_g) f32 PSUM recurrent gates, or None at step 0
            xsl: [128, GB] bf16 input-projection slice (includes bias)
            h_out: [128, 64] destination AP for the new hidden state
            returns the new cell state tile [128, 64] f32
            """
            gpool, spool, dpool, cpool = pools
            if ps is None:
                gi_ap = xsl[:, :6 * BC]
                gg_ap = xsl[:, 6 * BC:8 * BC]
            else:
                ps_ifo, ps_g = ps
                g_ifo = gpool.tile([128, 6 * BC], F32, tag="g_ifo")
                nc.vector.tensor_add(g_ifo[:], ps_ifo[:], xsl[:, :6 * BC])
                g_g = gpool.tile([128, 2 * BC], F32, tag="g_g")
                nc.vector.tensor_add(g_g[:], ps_g[:], xsl[:, 6 * BC:])
                gi_ap = g_ifo[:]
                gg_ap = g_g[:]
            sg = spool.tile([128, 6 * BC], BF16, tag="sg")
            nc.scalar.activation(sg[:], gi_ap, AF.Sigmoid)
            tg = spool.tile([128, 2 * BC], BF16, tag="tg")
            nc.scalar.activation(tg[:], gg_ap, AF.Tanh)
            c_new = cpool.tile([128, 2 * BC], F32)
            if c_prev is None:
                t1 = dpool.tile([128, 2 * BC], BF16, tag="t1")
                nc.vector.tensor_mul(t1[:], sg[:, :2 * BC], tg[:])
                nc.vector.tensor_copy(c_new[:], t1[:])
            else:
                u = dpool.tile([128, 2 * BC], F32, tag="u")
                nc.vector.tensor_mul(u[:], sg[:, 2 * BC:4 * BC], c_prev[:])
                t1 = dpool.tile([128, 2 * BC], BF16, tag="t1")
                nc.vector.tensor_mul(t1[:], sg[:, :2 * BC], tg[:])
                nc.vector.tensor_add(c_new[:], u[:], t1[:])
            tc_ = dpool.tile([128, 2 * BC], BF16, tag="tc")
            nc.scalar.activation(tc_[:], c_new[:], AF.Tanh)
            nc.vector.tensor_mul(h_out, sg[:, 4 * BC:6 * BC], tc_[:])
            return c_new

        # ============================================ phase A
        with ExitStack() as actx:
            gpoolA = actx.enter_context(tc.tile_pool(name="gatesA", bufs=3))
            spoolA = actx.enter_context(tc.tile_pool(name="sgA", bufs=3))
            dpoolA = actx.enter_context(tc.tile_pool(name="dA", bufs=3))
            cpoolA = actx.enter_context(tc.tile_pool(name="cA", bufs=4))
            rpsumA = actx.enter_context(
                tc.tile_pool(name="rpsA", bufs=2, space="PSUM"))
            cellA = (gpoolA, spoolA, dpoolA, cpoolA)

            gath = actx.enter_context(tc.tile_pool(name="gath", bufs=3))
            tpsum = actx.enter_context(
                tc.tile_pool(name="tps", bufs=2, space="PSUM"))
            xppool = actx.enter_context(
                tc.tile_pool(name="xpps", bufs=2, space="PSUM"))
            fspans = actx.enter_context(tc.tile_pool(name="fspan", bufs=2))
            bspans = actx.enter_context(tc.tile_pool(name="bspan", bufs=2))
            p1stg = actx.enter_context(tc.tile_pool(name="p1stg", bufs=2))

            xt = const.tile([128, 2, T * BC], BF16)

            chunks_done = set()

            def produce_chunks(span):
                for g in range(4 * span, 4 * span + 4):
                    if g in chunks_done:
                        continue
                    chunks_done.add(g)
                    gt = gath.tile([128, E], F32, tag="graw")
                    nc.gpsimd.indirect_dma_start(
                        out=gt[:], out_offset=None, in_=emb_d.ap(),
                        in_offset=IndirectOffsetOnAxis(
                            ap=idx_sb[:, g:g + 1], axis=0))
                    bt = gath.tile([128, E], BF16, tag="gbf")
                    nc.vector.tensor_copy(bt[:], gt[:])
                    for k in range(2):
                        tp = tpsum.tile([128, 128], BF16)
                        nc.tensor.transpose(
                            tp[:], bt[:, 128 * k:128 * (k + 1)], ident[:])
                        nc.vector.tensor_copy(
                            xt[:, k, 128 * g:128 * (g + 1)], tp[:])

            def produce_span_l0(pool, wih, bias, s):
                span = pool.tile([128, SPC], BF16)
                sv = span[:].rearrange("p (t m u) -> p t m u", m=8, u=BC)
                for m in range(8):
                    ps = xppool.tile([128, 512], F32)
                    for k in range(2):
                        nc.tensor.matmul(
                            ps[:],
                            lhsT=wih[:, k, 128 * m:128 * (m + 1)],
                            rhs=xt[:, k, 512 * s:512 * (s + 1)],
                            start=(k == 0), stop=(k == 1))
                    nc.scalar.activation(
                        sv[:, :, m, :],
                        ps[:].rearrange("p (t u) -> p t u", u=BC),
                        AF.Identity, bias=bias[:, m:m + 1])
                return span

            vyf = y0f[:].rearrange("p (t k u) -> p t k u", k=2, u=BC)

            def produce_p1f_span(blk):
                stg = p1stg.tile([128, SPC], BF16)
                svv = stg[:].rearrange("p (t m u) -> p t m u", m=8, u=BC)
                for m in range(8):
                    ps = xppool.tile([128, 512], F32)
                    for k in range(2):
                        nc.tensor.matmul(
                            ps[:],
                            lhsT=wsb["w_ih1f"][:, k, 128 * m:128 * (m + 1)],
                            rhs=vyf[:, 16 * blk:16 * (blk + 1), k, :],
                            start=(k == 0), stop=(k == 1))
                    nc.scalar.activation(
                        svv[:, :, m, :],
                        ps[:].rearrange("p (t u) -> p t u", u=BC),
                        AF.Identity, bias=wsb["b1f"][:, m:m + 1])
                nc.sync.dma_start(
                    out=p1f_dram.ap()[:, SPC * blk:SPC * (blk + 1)],
                    in_=stg[:])

            produce_chunks(0)
            if SP > 1:
                produce_chunks(SP - 1)
            fcur = produce_span_l0(fspans, wsb["w_ih0f"], wsb["b0f"], 0)
            bcur = produce_span_l0(bspans, wsb["w_ih0b"], wsb["b0b"], SP - 1)

            cf = cb = None
            hf_prev = hb_prev = None
            for blk in range(SP):
                if blk + 1 < SP:
                    produce_chunks(blk + 1)
                    produce_chunks(SP - 2 - blk)
                    fnext = produce_span_l0(
                        fspans, wsb["w_ih0f"], wsb["b0f"], blk + 1)
                    bnext = produce_span_l0(
                        bspans, wsb["w_ih0b"], wsb["b0b"], SP - 2 - blk)
                for toff in range(16):
                    t = 16 * blk + toff
                    tp_ = T - 1 - t
                    # layer-0 forward, time t
                    fsl = fcur[:, GB * toff:GB * (toff + 1)]
                    ps = emit_recur(rpsumA, wsb["w_hh0f"], hf_prev, fsl) \
                        if t > 0 else None
                    h_out = y0f[:, 64 * t:64 * (t + 1)]
                    cf = emit_cell(cellA, ps, fsl, cf, h_out)
                    hf_prev = h_out
                    # layer-0 backward, time position tp_
                    bsl = bcur[:, GB * (15 - toff):GB * (16 - toff)]
                    ps = emit_recur(rpsumA, wsb["w_hh0b"], hb_prev, bsl) \
                        if t > 0 else None
                    h_out = y0b[:, 64 * tp_:64 * (tp_ + 1)]
                    cb = emit_cell(cellA, ps, bsl, cb, h_out)
                    hb_prev = h_out
                produce_p1f_span(blk)
                if blk + 1 < SP:
                    fcur, bcur = fnext, bnext

        # ============================================ phase B
        with ExitStack() as bctx:
            gpoolB = bctx.enter_context(tc.tile_pool(name="gatesB", bufs=3))
            spoolB = bctx.enter_context(tc.tile_pool(name="sgB", bufs=3))
            dpoolB = bctx.enter_context(tc.tile_pool(name="dB", bufs=3))
            cpoolB = bctx.enter_context(tc.tile_pool(name="cB", bufs=4))
            h1pool = bctx.enter_context(tc.tile_pool(name="h1", bufs=3))
            rpsumB = bctx.enter_context(
                tc.tile_pool(name="rpsB", bufs=1, space="PSUM"))
            bppsum = bctx.enter_context(
                tc.tile_pool(name="bpps", bufs=1, space="PSUM"))
            finpool = bctx.enter_context(tc.tile_pool(name="fin", bufs=2))
            mgpool = bctx.enter_context(tc.tile_pool(name="mg", bufs=2))
            cellB = (gpoolB, spoolB, dpoolB, cpoolB)

            vyb = y0b[:].rearrange("p (t k u) -> p t k u", k=2, u=BC)

            def produce_merged(blk):
                fin = finpool.tile([128, SPC], BF16)
                nc.sync.dma_start(
                    out=fin[:],
                    in_=p1f_dram.ap()[:, SPC * blk:SPC * (blk + 1)])
                mg = mgpool.tile([128, SPC], BF16)
                for half in range(2):
                    ps = bppsum.tile([128, 8, 8 * BC], F32)
                    hs0 = 16 * blk + 8 * half
                    for m in range(8):
                        for k in range(2):
                            nc.tensor.matmul(
                                ps[:, m, :],
                                lhsT=wsb["w_ih1f"][:, 2 + k,
                                                   128 * m:128 * (m + 1)],
                                rhs=vyb[:, hs0:hs0 + 8, k, :],
                                start=(k == 0), stop=(k == 1))
                    half_sl = slice(SPC // 2 * half, SPC // 2 * (half + 1))
                    with nc.allow_low_precision("xproj merge in bf16"):
                        nc.vector.tensor_add(
                            mg[:, half_sl].rearrange(
                                "p (t m u) -> p t m u", m=8, u=BC),
                            fin[:, half_sl].rearrange(
                                "p (t m u) -> p t m u", m=8, u=BC),
                            ps[:].rearrange("p m (t u) -> p t m u", u=BC))
                return mg

            mcur = produce_merged(0)
            c1 = None
            h1_prev = None
            for blk in range(SP):
                if blk + 1 < SP:
                    mnext = produce_merged(blk + 1)
                for toff in range(16):
                    t = 16 * blk + toff
                    msl = mcur[:, GB * toff:GB * (toff + 1)]
                    ps = emit_recur(rpsumB, wsb["w_hh1f"], h1_prev, msl) \
                        if t > 0 else None
                    if t == T - 1:
                        h_out = out_sb[:, 0:64]
                    else:
                        h1 = h1pool.tile([128, 64], BF16)
                        h_out = h1[:]
                    c1 = emit_cell(cellB, ps, msl, c1, h_out)
                    h1_prev = h_out
                if blk + 1 < SP:
                    mcur = mnext

            # single layer-1 backward step (output position T-1, zero state)
            ps = rpsumB.tile([128, GB], F32, bufs=1)
            for m in range(8):
                for k in range(4):
                    src = y0f if k < 2 else y0b
                    kk = k % 2
                    nc.tensor.matmul(
                        ps[:, BC * m:BC * (m + 1)],
                        lhsT=wsb["w_ih1b"][:, k, 128 * m:128 * (m + 1)],
                        rhs=src[:, 64 * (T - 1) + 32 * kk:
                                64 * (T - 1) + 32 * (kk + 1)],
                        start=(k == 0), stop=(k == 3))
            g1b = gpoolB.tile([128, GB], F32)
            for m in range(8):
                nc.scalar.activation(
                    g1b[:, BC * m:BC * (m + 1)], ps[:, BC * m:BC * (m + 1)],
                    AF.Identity, bias=wsb["b1b"][:, m:m + 1])
            emit_cell(cellB, None, g1b[:], None, out_sb[:, 64:128])

            nc.sync.dma_start(out=out_d.ap(), in_=out_sb[:])

    nc.compile()
    return nc


# ---------------------------------------------------------------- entry

_CACHE = {}
_last_res = None


def _get_program(T):
    if T not in _CACHE:
        _CACHE[T] = build_program(T)
    return _CACHE[T]


def kernel(**inputs):
    return _kernel_impl(inputs, T_FULL)


def _kernel_impl(inputs, T):
    idc = np.asarray(inputs["utterances_idc"]).astype(np.int32)
    emb = np.asarray(inputs["emb"], np.float32)
    w = _prep_weights(inputs)

    nc = _get_program(T)

    in_maps = []
    for c in range(N_CORES):
        idc_c = idc[c * BC:(c + 1) * BC, :T]
        m = {"idxr": _reorder_idx(idc_c, T), "emb": emb}
        m.update(w)
        in_maps.append(m)

    global _last_res
    res = run_bass_kernel_spmd(nc, in_maps, list(range(N_CORES)))
    _last_res = res

    outs = []
    for c in range(N_CORES):
        o = res.results[c]["out"]  # [128, 4*BC]: [h1f k0,k1 | h1b k0,k1] x u
        o4 = o.reshape(128, 4, BC)
        # h1f dims: k-tile major -> [256, BC]; same for h1b
        h1f = np.concatenate([o4[:, 0, :], o4[:, 1, :]], 0)  # [256, BC]
        h1b = np.concatenate([o4[:, 2, :], o4[:, 3, :]], 0)
        outs.append(np.concatenate([h1f.T, h1b.T], 1))  # [BC, 512]
    return np.concatenate(outs, 0).astype(np.float32)
